# revision 18
# baseline (speedup 1.0000x reference)
"""Causal self-attention with RoPE on 8 Trainium2 NeuronCores.

Problem: B=4, T=2048, C=1024, NH=16, D=64. y = proj(attn(rope(qkv(x)))).

Sharding: core = (batch b, head-group hg): 4 batches x 2 groups of 8 heads.
Each core computes its 8 heads' attention for its batch plus the partial
output projection over its 512 head-channels; the host sums the two
partials per batch and adds b_proj.

On-device layout is "transposed" throughout ([feature partitions, token
free-dim]) so no on-chip transposes are needed:
  - qT/kT produced as [d, t] directly from the QKV matmul (bf16)
  - RoPE rotate_half via SBUF->SBUF DMA partition shuffle + sign-baked sin
  - scoresT[kv, q] = kT.T-slice @ qT-slice; the two heads of a pair run as
    concurrent row-tiled K=64 matmuls (base partitions 0/64)
  - softmax numerators exp(s/sqrt(d) - 2) on ScalarE; the -2 shift keeps
    values < 55 so fp8e4 (max 240) holds them for the DoubleRow PV path
  - off-diagonal kv tiles: PV in fp8e4 DoubleRow over kv-tile pairs (2x);
    diagonal band stays bf16 with the triangular mask
  - softmax denominator via a ones-column appended to V (free on PE)
  - output projection consumes normalized yT tiles as the stationary operand
Next-group QKV/V matmuls and previous-group projection chunks are emitted
between attention pairs so the PE stays fed while ScalarE grinds the exps
(the attention phase is exp-bound).
"""
import math
from contextlib import ExitStack

import numpy as np
import ml_dtypes

import concourse.bass as bass
import concourse.tile as tile
from concourse import bacc, mybir
from concourse.bass_utils import run_bass_kernel_spmd

B, T, C, NH, D = 4, 2048, 1024, 16, 64
P = 128                 # partitions
GN = 512                # token-group size
TG = T // GN            # 4 token groups
KT = C // P             # 8 contraction tiles over C
NCORES = 8
HPC = 8                 # heads per core
VS = 80                 # vplus8 per-head stride (16B-aligned fp8 slices)
f32 = mybir.dt.float32
bf16 = mybir.dt.bfloat16
f16 = mybir.dt.float16
fp8 = mybir.dt.float8e4
AF = mybir.ActivationFunctionType
DR = mybir.MatmulPerfMode.DoubleRow
BF = ml_dtypes.bfloat16
EXP_BIAS = -4.5         # exp(s/sqrt(d) - 4.5): keeps numerators < 240 for
                        # logits up to ~10 (TRN fp8e4 NaNs above 240)

_NC_CACHE = None


def _body(ctx, tc, xT, wqkT, wvT, wpT, bqk, bv, cosT, sinT, dmask,
          onescol, outp):
    nc = tc.nc

    const = ctx.enter_context(tc.tile_pool(name="const", bufs=1))
    resid = ctx.enter_context(tc.tile_pool(name="resid", bufs=1))
    xpool = ctx.enter_context(tc.tile_pool(name="xpool", bufs=16))
    cspool = ctx.enter_context(tc.tile_pool(name="cspool", bufs=2))
    rawp = ctx.enter_context(tc.tile_pool(name="rawp", bufs=2))
    shfp = ctx.enter_context(tc.tile_pool(name="shfp", bufs=2))
    tmpp = ctx.enter_context(tc.tile_pool(name="tmpp", bufs=2))
    attp = ctx.enter_context(tc.tile_pool(name="attp", bufs=3))
    a8p = ctx.enter_context(tc.tile_pool(name="a8p", bufs=2))
    bcp = ctx.enter_context(tc.tile_pool(name="bcp", bufs=2))
    rcp = ctx.enter_context(tc.tile_pool(name="rcp", bufs=2))
    rsp = ctx.enter_context(tc.tile_pool(name="rsp", bufs=2))
    outsb = ctx.enter_context(tc.tile_pool(name="outsb", bufs=2))
    psmm = ctx.enter_context(tc.tile_pool(name="psmm", bufs=2, space="PSUM"))
    pssc = ctx.enter_context(tc.tile_pool(name="pssc", bufs=2, space="PSUM"))
    psy = ctx.enter_context(tc.tile_pool(name="psy", bufs=2, space="PSUM"))

    # ---- constants / resident tensors ----
    # wqk and the g=0 x tiles interleaved so the first QKV matmul isn't
    # gated on the whole weight load.
    wqk_t = []
    x0_t = []
    for k in range(KT):
        w_ = const.tile([P, 1024], bf16, tag=f"wqk{k}", name=f"wqk{k}")
        nc.sync.dma_start(w_[:], wqkT[k * P:(k + 1) * P, :])
        wqk_t.append(w_)
        x_ = xpool.tile([P, GN], bf16, tag="xt", name=f"xt0_{k}")
        nc.sync.dma_start(x_[:], xT[k * P:(k + 1) * P, 0:GN])
        x0_t.append(x_)
    wv_t = [const.tile([P, 512], bf16, tag=f"wv{k}", name=f"wv{k}")
            for k in range(KT)]
    wp_t = [const.tile([P, 1024], bf16, tag=f"wp{k}", name=f"wp{k}")
            for k in range(4)]
    dmask_t = const.tile([P, P], bf16, tag="dmask", name="dmask_t")
    nc.sync.dma_start(dmask_t[:], dmask[:])
    bqk_t = const.tile([P, 8], f32, tag="bqk", name="bqk_t")
    nc.sync.dma_start(bqk_t[:], bqk[:])
    bv_t = const.tile([1, 512], bf16, tag="bv", name="bv_t")
    nc.sync.dma_start(bv_t[:], bv[:])
    ones_t = const.tile([P, P], bf16, tag="ones", name="ones_t")
    nc.sync.dma_start(ones_t[:], onescol[:])
    ebias_t = const.tile([P, 1], f32, tag="ebias", name="ebias_t")
    nc.gpsimd.memset(ebias_t[:], EXP_BIAS)

    kT_t = [resid.tile([P, T], bf16, tag=f"kT{p}", name=f"kT{p}")
            for p in range(4)]
    # bf16 V (+ones) for the diagonal band, fp8 V (+ones) for DoubleRow.
    vplus = resid.tile([P, 16 * HPC * 72], bf16, tag="vplus", name="vplus")
    vp4 = vplus[:].rearrange("p (t h e) -> p t h e", t=16, h=HPC)
    nc.gpsimd.memset(vp4[:, :, :, 64:65], 1.0)
    vplus8 = resid.tile([P, 16 * HPC * VS], fp8, tag="vplus8", name="vplus8")
    vp8 = vplus8[:].rearrange("p (t h e) -> p t h e", t=16, h=HPC)
    nc.gpsimd.memset(vp8[:, :, :, 64:65], 1.0)
    # double-buffered qT/yT so next-group QKV and prev-group proj can
    # interleave with the current attention phase
    qT_g = [[resid.tile([P, GN], bf16, tag=f"qT{b_}_{p}", name=f"qT{b_}_{p}")
             for p in range(4)] for b_ in range(2)]
    yT_g = [[resid.tile([P, GN], bf16, tag=f"yT{b_}_{p}", name=f"yT{b_}_{p}")
             for p in range(4)] for b_ in range(2)]

    def _loads(g_):
        """DMA loads for group g_ (cos/sin/x)."""
        cos_t = cspool.tile([P, GN], bf16, tag="cos", name=f"cos{g_}")
        nc.sync.dma_start(cos_t[:], cosT[:, g_ * GN:(g_ + 1) * GN])
        sin_t = cspool.tile([P, GN], bf16, tag="sin", name=f"sin{g_}")
        nc.sync.dma_start(sin_t[:], sinT[:, g_ * GN:(g_ + 1) * GN])
        if g_ == 0:
            x_t = x0_t
        else:
            x_t = []
            for k in range(KT):
                x_ = xpool.tile([P, GN], bf16, tag="xt", name=f"xt{g_}_{k}")
                nc.sync.dma_start(x_[:], xT[k * P:(k + 1) * P,
                                            g_ * GN:(g_ + 1) * GN])
                x_t.append(x_)
        return {"cos": cos_t, "sin": sin_t, "x": x_t}

    def _fstep(g_, f, ld):
        """One QKV q/k feature tile (128 feats) + RoPE for group g_."""
        mm_ps = psmm.tile([P, GN], f32, tag="mm", name=f"qkps{g_}_{f}")
        for k in range(KT):
            nc.tensor.matmul(mm_ps[:], wqk_t[k][:, f * P:(f + 1) * P],
                             ld["x"][k][:], start=(k == 0), stop=(k == KT - 1))
        raw = rawp.tile([P, GN], bf16, tag="raw", name=f"raw{g_}_{f}")
        nc.vector.tensor_scalar_add(raw[:], mm_ps[:], bqk_t[:, f:f + 1])
        # rotate_half via partition shuffle (SBUF->SBUF DMA); the sign of
        # the second half is baked into the sin table host-side.
        shuf = shfp.tile([P, GN], bf16, tag="shuf", name=f"shuf{g_}_{f}")
        for hb in (0, 64):
            nc.sync.dma_start(shuf[hb:hb + 32, :], raw[hb + 32:hb + 64, :])
            nc.sync.dma_start(shuf[hb + 32:hb + 64, :], raw[hb:hb + 32, :])
        dst = qT_g[g_ % 2][f][:] if f < 4 else kT_t[f - 4][:, g_ * GN:(g_ + 1) * GN]
        nc.vector.tensor_mul(dst, raw[:], ld["cos"][:])
        tmp = tmpp.tile([P, GN], bf16, tag="tmp", name=f"tmp{g_}_{f}")
        nc.vector.tensor_mul(tmp[:], shuf[:], ld["sin"][:])
        nc.vector.tensor_add(dst, dst, tmp[:])

    def _vstep(g_, tt, ld):
        """One V token tile (128 tokens) for group g_, bf16 + fp8 copies."""
        ttg = g_ * 4 + tt
        v_ps = psmm.tile([P, GN], f32, tag="mm", name=f"vps{g_}_{tt}")
        for k in range(KT):
            nc.tensor.matmul(v_ps[:], ld["x"][k][:, tt * P:(tt + 1) * P],
                             wv_t[k][:], start=(k == 0), stop=False)
        nc.tensor.matmul(v_ps[:], ones_t[0:1, :], bv_t[:],
                         start=False, stop=True)
        vh = v_ps[:].rearrange("p (h e) -> p h e", h=HPC)
        nc.vector.tensor_copy(vp4[:, ttg, :, 0:64], vh)
        nc.gpsimd.tensor_copy(vp8[:, ttg, :, 0:64], vp4[:, ttg, :, 0:64])

    def _proj_chunk(g_, tt, n, pool, p3_last=False):
        """One output-projection chunk (128 tokens x 512 channels)."""
        o_ps = pool.tile([P, GN], f32, tag="mm" if pool is psmm else "y",
                         name=f"ops{g_}_{tt}_{n}")
        yT = yT_g[g_ % 2]
        order = (0, 1, 2, 3)
        for i, p in enumerate(order):
            nc.tensor.matmul(o_ps[:], yT[p][:, tt * P:(tt + 1) * P],
                             wp_t[p][:, n * GN:(n + 1) * GN],
                             start=(i == 0), stop=(i == 3))
        o_sb = outsb.tile([P, GN], f16, tag="osb", name=f"osb{g_}_{tt}_{n}")
        nc.vector.tensor_copy(o_sb[:], o_ps[:])
        nc.sync.dma_start(
            outp[g_ * GN + tt * P: g_ * GN + (tt + 1) * P,
                 n * GN:(n + 1) * GN], o_sb[:])

    def _proj_fin(g_, tn, o_ps):
        tt, n = tn
        yTl = yT_g[g_ % 2]
        nc.tensor.matmul(o_ps[:], yTl[3][:, tt * P:(tt + 1) * P],
                         wp_t[3][:, n * GN:(n + 1) * GN],
                         start=False, stop=True)
        o_sb = outsb.tile([P, GN], f16, tag="osb", name=f"osbF{g_}_{tt}_{n}")
        nc.vector.tensor_copy(o_sb[:], o_ps[:])
        nc.sync.dma_start(
            outp[g_ * GN + tt * P: g_ * GN + (tt + 1) * P,
                 n * GN:(n + 1) * GN], o_sb[:])

    def _attention(g, work):
        """Attention pairs for group g; `work` is a deque of closures
        emitting interleaved PE work (next-group QKV / prev-group proj),
        paced one item per attention unit to avoid engine-queue bursts."""
        njt = 4 * g + 4
        qT = qT_g[g % 2]
        yT = yT_g[g % 2]

        def _finish_norm(p_, rcrows, is_f32):
            bdt = f32 if is_f32 else bf16
            bcb = bcp.tile([P, GN], bdt, tag="bcb" + ("f" if is_f32 else ""),
                           name=f"bcb{g}_{p_}")
            nc.gpsimd.partition_broadcast(bcb[0:64, :], rcrows[0][:])
            nc.vector.tensor_mul(yT[p_][0:64, :], yT[p_][0:64, :],
                                 bcb[0:64, :])
            bcb2 = bcp.tile([P, GN], bdt, tag="bcb" + ("f" if is_f32 else ""),
                            name=f"bcb2{g}_{p_}")
            nc.gpsimd.partition_broadcast(bcb2[0:64, :], rcrows[1][:])
            nc.sync.dma_start(bcb2[64:128, :], bcb2[0:64, :])
            nc.vector.tensor_mul(yT[p_][64:128, :],
                                 yT[p_][64:128, :], bcb2[64:128, :])

        pending_norm = None
        for p in range(4):
            yps = [psy.tile([65, GN], f32, tag="y", name=f"yps{g}_{p}_{s}")
                   for s in range(2)]
            # units: 4 diagonal tiles first (their mask latency hides
            # behind later work), then off-diagonal kv tile pairs with
            # fp8 DoubleRow PV.
            units = [("diag", j) for j in range(4 * g, njt)] + \
                    [("pair", m) for m in range(2 * g)]
            nu = len(units)
            prev = None

            def _pv(ui):
                kind, arg = units[ui]
                a_ = prev
                if kind == "diag":
                    for s in range(2):
                        nc.tensor.matmul(yps[s][:], vp4[:, arg, 2 * p + s, 0:65],
                                         a_[:, s * GN:(s + 1) * GN],
                                         start=(ui == 0), stop=(ui == nu - 1))
                else:
                    a8v = a_[:].rearrange("p (j x) -> p j x", j=2)
                    for s in range(2):
                        nc.tensor.matmul(
                            yps[s][:],
                            vp8[:, 2 * arg:2 * arg + 2, 2 * p + s, 0:65],
                            a8v[:, :, s * GN:(s + 1) * GN],
                            start=(ui == 0), stop=(ui == nu - 1),
                            perf_mode=DR, skip_group_check=True)

            for ui, (kind, arg) in enumerate(units):
                if kind == "diag":
                    j = arg
                    r = j - 4 * g
                    c0 = r * P
                    sc2 = pssc.tile([P, 2 * GN], f32, tag="sc",
                                    name=f"sc{g}_{p}_{j}")
                    for s in range(2):
                        hb = s * 64
                        nc.tensor.matmul(
                            sc2[:, s * GN + c0:(s + 1) * GN],
                            kT_t[p][hb:hb + 64, j * P:(j + 1) * P],
                            qT[p][hb:hb + 64, c0:GN],
                            start=True, stop=True)
                    a2 = attp.tile([P, 2 * GN], bf16, tag="att",
                                   name=f"att{g}_{p}_{j}")
                    sc2v = sc2[:].rearrange("p (s q) -> p s q", s=2)
                    a2v = a2[:].rearrange("p (s q) -> p s q", s=2)
                    if c0 > 0:
                        nc.gpsimd.memset(a2v[:, :, 0:c0], 0.0)
                    nc.scalar.activation(a2v[:, :, c0:GN], sc2v[:, :, c0:GN],
                                         AF.Exp, scale=1.0 / math.sqrt(D),
                                         bias=ebias_t[:, 0:1])
                    nc.vector.tensor_mul(a2[:, c0:c0 + P],
                                         a2[:, c0:c0 + P], dmask_t[:])
                    nc.vector.tensor_mul(a2[:, GN + c0:GN + c0 + P],
                                         a2[:, GN + c0:GN + c0 + P], dmask_t[:])
                    nxt = a2
                else:
                    m = arg
                    a8 = a8p.tile([P, 2 * 2 * GN], fp8, tag="a8",
                                  name=f"a8_{g}_{p}_{m}")
                    a8v = a8[:].rearrange("p (j x) -> p j x", j=2)
                    for jj in range(2):
                        j = 2 * m + jj
                        sc2 = pssc.tile([P, 2 * GN], f32, tag="sc",
                                        name=f"sc{g}_{p}_{j}")
                        for s in range(2):
                            hb = s * 64
                            nc.tensor.matmul(
                                sc2[:, s * GN:(s + 1) * GN],
                                kT_t[p][hb:hb + 64, j * P:(j + 1) * P],
                                qT[p][hb:hb + 64, :],
                                start=True, stop=True)
                        nc.scalar.activation(a8v[:, jj, :], sc2[:],
                                             AF.Exp, scale=1.0 / math.sqrt(D),
                                             bias=ebias_t[:, 0:1])
                    nxt = a8
                if ui > 0:
                    _pv(ui - 1)
                    prev = nxt
                else:
                    prev = nxt
                if work:
                    work.popleft()()
            _pv(nu - 1)
            if pending_norm is not None:
                _finish_norm(*pending_norm)
            rcrows = []
            rs_p = rsp.tile([P, 8], f32, tag="rs", name=f"rs{g}_{p}")
            for s in range(2):
                nc.vector.tensor_copy(yT[p][s * 64:(s + 1) * 64, :],
                                      yps[s][0:64, :])
                rrow = rcp.tile([1, GN], f32, tag="rrow",
                                name=f"rrow{g}_{p}_{s}")
                nc.vector.tensor_copy(rrow[:], yps[s][64:65, :])
                nc.sync.dma_start(rs_p[:, s * 4:(s + 1) * 4], rrow[:])
            rc_p = rsp.tile([P, 8], f32, tag="rc", name=f"rcp{g}_{p}")
            nc.vector.reciprocal(rc_p[:], rs_p[:])
            rcb = rsp.tile([P, 8], bf16, tag="rcb", name=f"rcb{g}_{p}")
            nc.vector.tensor_copy(rcb[:], rc_p[:])
            for s in range(2):
                rcrow = rcp.tile([1, GN], bf16, tag="rcrow", bufs=2,
                                 name=f"rcrow{g}_{p}_{s}")
                nc.sync.dma_start(rcrow[:], rcb[:, s * 4:(s + 1) * 4])
                rcrows.append(rcrow)
            pending_norm = (p, rcrows, False)
        while work:
            work.popleft()()
        _finish_norm(*pending_norm)

    # ================= main schedule =================
    from collections import deque

    ld = _loads(0)
    wv_loaded = [False]

    def _load_wvp():
        for k in range(KT):
            nc.sync.dma_start(wv_t[k][:], wvT[k * P:(k + 1) * P, :])
        for k in range(4):
            nc.sync.dma_start(wp_t[k][:], wpT[k * P:(k + 1) * P, :])

    _load_wvp()
    # g0 QKV emitted so attention(0) pair p's operands (f=p q-feats,
    # f=4+p k-feats) and V tiles complete early
    for p in range(4):
        _fstep(0, p, ld)
        _fstep(0, 4 + p, ld)
        _vstep(0, p, ld)

    next_ld = [None]

    def _qkv_items(g_, nl):
        items = []
        for p in range(4):
            items.append(lambda p=p: _fstep(g_, p, nl))
            items.append(lambda p=p: _fstep(g_, 4 + p, nl))
            items.append(lambda p=p: _vstep(g_, p, nl))
        return items

    for g in range(TG):
        work = deque()
        if g < TG - 1:
            next_ld[0] = _loads(g + 1)
            work.extend(_qkv_items(g + 1, next_ld[0]))
        if g >= 1:
            # previous group's projection, interleaved round-robin
            pitems = [lambda tt=tt, n=n: _proj_chunk(g - 1, tt, n, psmm)
                      for tt in range(4) for n in range(2)]
            mixed = deque()
            wl = list(work)
            i = j = 0
            while i < len(wl) or j < len(pitems):
                if i < len(wl):
                    mixed.append(wl[i]); i += 1
                    if i % 3 == 0 and j < len(pitems):
                        mixed.append(pitems[j]); j += 1
                else:
                    mixed.append(pitems[j]); j += 1
            work = mixed
        _attention(g, work)

    # final projection (g3): 2-deep pipeline with the p=3 matmul last so
    # the first chunks' p0-p2 matmuls run under the last pair's norm chain
    g3 = TG - 1
    chunks = [(tt, n) for tt in range(4) for n in range(2)]
    o_tiles = {}
    for ci, (tt, n) in enumerate(chunks):
        o_ps = psy.tile([P, GN], f32, tag="y", name=f"opsF_{tt}_{n}")
        o_tiles[ci] = o_ps
        yTl = yT_g[g3 % 2]
        for i, p in enumerate((0, 1, 2)):
            nc.tensor.matmul(o_ps[:], yTl[p][:, tt * P:(tt + 1) * P],
                             wp_t[p][:, n * GN:(n + 1) * GN],
                             start=(i == 0), stop=False)
        if ci >= 1:
            _proj_fin(g3, chunks[ci - 1], o_tiles.pop(ci - 1))
    _proj_fin(g3, chunks[-1], o_tiles.pop(len(chunks) - 1))


def build_nc():
    nc = bacc.Bacc("TRN2", target_bir_lowering=False, debug=False,
                   num_devices=NCORES)
    xT = nc.dram_tensor("xT", [C, T], bf16, kind="ExternalInput").ap()
    wqkT = nc.dram_tensor("wqkT", [C, 1024], bf16, kind="ExternalInput").ap()
    wvT = nc.dram_tensor("wvT", [C, 512], bf16, kind="ExternalInput").ap()
    wpT = nc.dram_tensor("wpT", [512, 1024], bf16, kind="ExternalInput").ap()
    bqk = nc.dram_tensor("bqk", [P, 8], f32, kind="ExternalInput").ap()
    bv = nc.dram_tensor("bv", [1, 512], bf16, kind="ExternalInput").ap()
    cosT = nc.dram_tensor("cosT", [P, T], bf16, kind="ExternalInput").ap()
    sinT = nc.dram_tensor("sinT", [P, T], bf16, kind="ExternalInput").ap()
    dmask = nc.dram_tensor("dmask", [P, P], bf16, kind="ExternalInput").ap()
    onescol = nc.dram_tensor("onescol", [P, P], bf16, kind="ExternalInput").ap()
    outp = nc.dram_tensor("outp", [T, C], f16, kind="ExternalOutput").ap()
    with tile.TileContext(nc) as tc, \
            nc.allow_low_precision(reason="bf16/fp8 matmul operands"):
        with ExitStack() as ctx:
            _body(ctx, tc, xT, wqkT, wvT, wpT, bqk, bv, cosT, sinT,
                  dmask, onescol, outp)
    nc.compile()
    return nc


def _host_inputs(x, w_attn, b_attn, w_proj, cos, sin):
    """Build the 8 per-core input dicts."""
    dmask = np.triu(np.ones((P, P), np.float32))
    onescol = np.ones((P, P), np.float32)
    cosT2 = np.ascontiguousarray(
        np.concatenate([cos[0].T, cos[0].T], axis=0))      # [128, T]
    sinT2 = np.concatenate([sin[0].T, sin[0].T], axis=0)
    # rotate_half sign baked into sin: rows d<32 of each 64-row block get
    # the minus (tmp[0:32] = -u2*sin, tmp[32:64] = +u1*sin)
    sgn = np.where((np.arange(P) % 64) < 32, -1.0, 1.0).astype(np.float32)
    sinT2 = np.ascontiguousarray(sinT2 * sgn[:, None])
    dmask = dmask.astype(BF)
    onescol = onescol.astype(BF)
    cosT2 = cosT2.astype(BF)
    sinT2 = sinT2.astype(BF)

    in_maps = []
    for core in range(NCORES):
        b = core // 2
        hg = core % 2
        h0 = hg * HPC
        qrows = slice(h0 * D, (h0 + HPC) * D)              # 512 rows
        krows = slice(C + h0 * D, C + (h0 + HPC) * D)
        vrows = slice(2 * C + h0 * D, 2 * C + (h0 + HPC) * D)
        wqk = np.concatenate([w_attn[qrows], w_attn[krows]], axis=0)  # [1024, C]
        bqk_np = np.concatenate([b_attn[qrows], b_attn[krows]])       # [1024]
        in_maps.append({
            "xT": np.ascontiguousarray(x[b].T).astype(BF),             # [C, T]
            "wqkT": np.ascontiguousarray(wqk.T).astype(BF),            # [C, 1024]
            "wvT": np.ascontiguousarray(w_attn[vrows].T).astype(BF),   # [C, 512]
            "wpT": np.ascontiguousarray(
                w_proj[:, h0 * D:(h0 + HPC) * D].T).astype(BF),
            "bqk": np.ascontiguousarray(bqk_np.reshape(8, P).T),       # [128, 8]
            "bv": np.ascontiguousarray(
                b_attn[vrows].reshape(1, 512)).astype(BF),
            "cosT": cosT2,
            "sinT": sinT2,
            "dmask": dmask,
            "onescol": onescol,
        })
    return in_maps


def kernel(x, w_attn, b_attn, w_proj, b_proj, cos, sin):
    global _NC_CACHE
    x = np.asarray(x, np.float32)
    w_attn = np.asarray(w_attn, np.float32)
    b_attn = np.asarray(b_attn, np.float32)
    w_proj = np.asarray(w_proj, np.float32)
    b_proj = np.asarray(b_proj, np.float32)
    cos = np.asarray(cos, np.float32)
    sin = np.asarray(sin, np.float32)

    if _NC_CACHE is None:
        _NC_CACHE = build_nc()
    nc = _NC_CACHE
    in_maps = _host_inputs(x, w_attn, b_attn, w_proj, cos, sin)
    res = run_bass_kernel_spmd(nc, in_maps, core_ids=list(range(NCORES)))
    parts = [res.results[i]["outp"] for i in range(NCORES)]
    out = np.empty((B, T, C), np.float32)
    for b in range(B):
        out[b] = (parts[2 * b].astype(np.float32)
                  + parts[2 * b + 1].astype(np.float32) + b_proj)
    return out


# revision 19
# speedup vs baseline: 1.0368x; 1.0368x over previous
"""Causal self-attention with RoPE on 8 Trainium2 NeuronCores.

Problem: B=4, T=2048, C=1024, NH=16, D=64. y = proj(attn(rope(qkv(x)))).

Sharding: core = (batch b, head-group hg): 4 batches x 2 groups of 8 heads.
Each core computes its 8 heads' attention for its batch plus the partial
output projection over its 512 head-channels; the host sums the two
partials per batch and adds b_proj.

On-device layout is "transposed" throughout ([feature partitions, token
free-dim]) so no on-chip transposes are needed:
  - qT/kT produced as [d, t] directly from the QKV matmul (bf16)
  - RoPE rotate_half via SBUF->SBUF DMA partition shuffle + sign-baked sin
  - scoresT[kv, q] = kT.T-slice @ qT-slice; the two heads of a pair run as
    concurrent row-tiled K=64 matmuls (base partitions 0/64)
  - softmax numerators exp(s/sqrt(d) - 2) on ScalarE; the -2 shift keeps
    values < 55 so fp8e4 (max 240) holds them for the DoubleRow PV path
  - off-diagonal kv tiles: PV in fp8e4 DoubleRow over kv-tile pairs (2x);
    diagonal band stays bf16 with the triangular mask
  - softmax denominator via a ones-column appended to V (free on PE)
  - output projection consumes normalized yT tiles as the stationary operand
Next-group QKV/V matmuls and previous-group projection chunks are emitted
between attention pairs so the PE stays fed while ScalarE grinds the exps
(the attention phase is exp-bound).
"""
import math
from contextlib import ExitStack

import numpy as np
import ml_dtypes

import concourse.bass as bass
import concourse.tile as tile
from concourse import bacc, mybir
from concourse.bass_utils import run_bass_kernel_spmd

B, T, C, NH, D = 4, 2048, 1024, 16, 64
P = 128                 # partitions
GN = 512                # token-group size
TG = T // GN            # 4 token groups
KT = C // P             # 8 contraction tiles over C
NCORES = 8
HPC = 8                 # heads per core
VS = 80                 # vplus8 per-head stride (16B-aligned fp8 slices)
f32 = mybir.dt.float32
bf16 = mybir.dt.bfloat16
f16 = mybir.dt.float16
fp8 = mybir.dt.float8e4
AF = mybir.ActivationFunctionType
DR = mybir.MatmulPerfMode.DoubleRow
BF = ml_dtypes.bfloat16
EXP_BIAS = -4.5         # exp(s/sqrt(d) - 4.5): keeps numerators < 240 for
                        # logits up to ~10 (TRN fp8e4 NaNs above 240)

_NC_CACHE = None


def _body(ctx, tc, xT, wqkT, wvT, wpT, bqk, bv, cosT, sinT, dmask,
          onescol, outp):
    nc = tc.nc

    const = ctx.enter_context(tc.tile_pool(name="const", bufs=1))
    resid = ctx.enter_context(tc.tile_pool(name="resid", bufs=1))
    xpool = ctx.enter_context(tc.tile_pool(name="xpool", bufs=16))
    cspool = ctx.enter_context(tc.tile_pool(name="cspool", bufs=2))
    rawp = ctx.enter_context(tc.tile_pool(name="rawp", bufs=2))
    shfp = ctx.enter_context(tc.tile_pool(name="shfp", bufs=2))
    tmpp = ctx.enter_context(tc.tile_pool(name="tmpp", bufs=2))
    attp = ctx.enter_context(tc.tile_pool(name="attp", bufs=3))
    a8p = ctx.enter_context(tc.tile_pool(name="a8p", bufs=2))
    bcp = ctx.enter_context(tc.tile_pool(name="bcp", bufs=2))
    rcp = ctx.enter_context(tc.tile_pool(name="rcp", bufs=2))
    rsp = ctx.enter_context(tc.tile_pool(name="rsp", bufs=2))
    outsb = ctx.enter_context(tc.tile_pool(name="outsb", bufs=2))
    psmm = ctx.enter_context(tc.tile_pool(name="psmm", bufs=2, space="PSUM"))
    pssc = ctx.enter_context(tc.tile_pool(name="pssc", bufs=2, space="PSUM"))
    psy = ctx.enter_context(tc.tile_pool(name="psy", bufs=2, space="PSUM"))

    # ---- constants / resident tensors ----
    # wqk and the g=0 x tiles interleaved so the first QKV matmul isn't
    # gated on the whole weight load.
    wqk_t = []
    x0_t = []
    for k in range(KT):
        w_ = const.tile([P, 1024], bf16, tag=f"wqk{k}", name=f"wqk{k}")
        nc.sync.dma_start(w_[:], wqkT[k * P:(k + 1) * P, :])
        wqk_t.append(w_)
        x_ = xpool.tile([P, GN], bf16, tag="xt", name=f"xt0_{k}")
        nc.sync.dma_start(x_[:], xT[k * P:(k + 1) * P, 0:GN])
        x0_t.append(x_)
    wv_t = [const.tile([P, 512], bf16, tag=f"wv{k}", name=f"wv{k}")
            for k in range(KT)]
    wp_t = [const.tile([P, 1024], bf16, tag=f"wp{k}", name=f"wp{k}")
            for k in range(4)]
    dmask_t = const.tile([P, P], bf16, tag="dmask", name="dmask_t")
    nc.sync.dma_start(dmask_t[:], dmask[:])
    bqk_t = const.tile([P, 8], f32, tag="bqk", name="bqk_t")
    nc.sync.dma_start(bqk_t[:], bqk[:])
    bv_t = const.tile([1, 512], bf16, tag="bv", name="bv_t")
    nc.sync.dma_start(bv_t[:], bv[:])
    ones_t = const.tile([P, P], bf16, tag="ones", name="ones_t")
    nc.sync.dma_start(ones_t[:], onescol[:])
    ebias_t = const.tile([P, 1], f32, tag="ebias", name="ebias_t")
    nc.gpsimd.memset(ebias_t[:], EXP_BIAS)

    kT_t = [resid.tile([P, T], bf16, tag=f"kT{p}", name=f"kT{p}")
            for p in range(4)]
    # bf16 V (+ones) for the diagonal band, fp8 V (+ones) for DoubleRow.
    vplus = resid.tile([P, 16 * HPC * 72], bf16, tag="vplus", name="vplus")
    vp4 = vplus[:].rearrange("p (t h e) -> p t h e", t=16, h=HPC)
    nc.gpsimd.memset(vp4[:, :, :, 64:65], 1.0)
    vplus8 = resid.tile([P, 16 * HPC * VS], fp8, tag="vplus8", name="vplus8")
    vp8 = vplus8[:].rearrange("p (t h e) -> p t h e", t=16, h=HPC)
    nc.gpsimd.memset(vp8[:, :, :, 64:65], 1.0)
    # double-buffered qT/yT so next-group QKV and prev-group proj can
    # interleave with the current attention phase
    qT_g = [[resid.tile([P, GN], bf16, tag=f"qT{b_}_{p}", name=f"qT{b_}_{p}")
             for p in range(4)] for b_ in range(2)]
    yT_g = [[resid.tile([P, GN], bf16, tag=f"yT{b_}_{p}", name=f"yT{b_}_{p}")
             for p in range(4)] for b_ in range(2)]

    def _loads(g_):
        """DMA loads for group g_ (cos/sin/x)."""
        cos_t = cspool.tile([P, GN], bf16, tag="cos", name=f"cos{g_}")
        nc.sync.dma_start(cos_t[:], cosT[:, g_ * GN:(g_ + 1) * GN])
        sin_t = cspool.tile([P, GN], bf16, tag="sin", name=f"sin{g_}")
        nc.sync.dma_start(sin_t[:], sinT[:, g_ * GN:(g_ + 1) * GN])
        if g_ == 0:
            x_t = x0_t
        else:
            x_t = []
            for k in range(KT):
                x_ = xpool.tile([P, GN], bf16, tag="xt", name=f"xt{g_}_{k}")
                nc.sync.dma_start(x_[:], xT[k * P:(k + 1) * P,
                                            g_ * GN:(g_ + 1) * GN])
                x_t.append(x_)
        return {"cos": cos_t, "sin": sin_t, "x": x_t}

    def _fstep(g_, f, ld):
        """One QKV q/k feature tile (128 feats) + RoPE for group g_."""
        mm_ps = psmm.tile([P, GN], f32, tag="mm", name=f"qkps{g_}_{f}")
        for k in range(KT):
            nc.tensor.matmul(mm_ps[:], wqk_t[k][:, f * P:(f + 1) * P],
                             ld["x"][k][:], start=(k == 0), stop=(k == KT - 1))
        raw = rawp.tile([P, GN], bf16, tag="raw", name=f"raw{g_}_{f}")
        nc.vector.tensor_scalar_add(raw[:], mm_ps[:], bqk_t[:, f:f + 1])
        # rotate_half via partition shuffle (SBUF->SBUF DMA); the sign of
        # the second half is baked into the sin table host-side.
        shuf = shfp.tile([P, GN], bf16, tag="shuf", name=f"shuf{g_}_{f}")
        for hb in (0, 64):
            nc.sync.dma_start(shuf[hb:hb + 32, :], raw[hb + 32:hb + 64, :])
            nc.sync.dma_start(shuf[hb + 32:hb + 64, :], raw[hb:hb + 32, :])
        dst = qT_g[g_ % 2][f][:] if f < 4 else kT_t[f - 4][:, g_ * GN:(g_ + 1) * GN]
        nc.vector.tensor_mul(dst, raw[:], ld["cos"][:])
        tmp = tmpp.tile([P, GN], bf16, tag="tmp", name=f"tmp{g_}_{f}")
        nc.vector.tensor_mul(tmp[:], shuf[:], ld["sin"][:])
        nc.vector.tensor_add(dst, dst, tmp[:])

    def _vstep(g_, tt, ld):
        """One V token tile (128 tokens) for group g_, bf16 + fp8 copies."""
        ttg = g_ * 4 + tt
        v_ps = psmm.tile([P, GN], f32, tag="mm", name=f"vps{g_}_{tt}")
        for k in range(KT):
            nc.tensor.matmul(v_ps[:], ld["x"][k][:, tt * P:(tt + 1) * P],
                             wv_t[k][:], start=(k == 0), stop=False)
        nc.tensor.matmul(v_ps[:], ones_t[0:1, :], bv_t[:],
                         start=False, stop=True)
        vh = v_ps[:].rearrange("p (h e) -> p h e", h=HPC)
        nc.vector.tensor_copy(vp4[:, ttg, :, 0:64], vh)
        nc.gpsimd.tensor_copy(vp8[:, ttg, :, 0:64], vp4[:, ttg, :, 0:64])

    def _proj_chunk(g_, tt, n, pool, p3_last=False):
        """One output-projection chunk (128 tokens x 512 channels)."""
        o_ps = pool.tile([P, GN], f32, tag="mm" if pool is psmm else "y",
                         name=f"ops{g_}_{tt}_{n}")
        yT = yT_g[g_ % 2]
        order = (0, 1, 2, 3)
        for i, p in enumerate(order):
            nc.tensor.matmul(o_ps[:], yT[p][:, tt * P:(tt + 1) * P],
                             wp_t[p][:, n * GN:(n + 1) * GN],
                             start=(i == 0), stop=(i == 3))
        o_sb = outsb.tile([P, GN], f16, tag="osb", name=f"osb{g_}_{tt}_{n}")
        nc.vector.tensor_copy(o_sb[:], o_ps[:])
        nc.sync.dma_start(
            outp[g_ * GN + tt * P: g_ * GN + (tt + 1) * P,
                 n * GN:(n + 1) * GN], o_sb[:])

    def _proj_fin(g_, tn, o_ps):
        tt, n = tn
        yTl = yT_g[g_ % 2]
        nc.tensor.matmul(o_ps[:], yTl[3][:, tt * P:(tt + 1) * P],
                         wp_t[3][:, n * GN:(n + 1) * GN],
                         start=False, stop=True)
        o_sb = outsb.tile([P, GN], f16, tag="osb", name=f"osbF{g_}_{tt}_{n}")
        nc.vector.tensor_copy(o_sb[:], o_ps[:])
        nc.sync.dma_start(
            outp[g_ * GN + tt * P: g_ * GN + (tt + 1) * P,
                 n * GN:(n + 1) * GN], o_sb[:])

    def _attention(g, work):
        """Attention pairs for group g; `work` is a deque of closures
        emitting interleaved PE work (next-group QKV / prev-group proj),
        paced one item per attention unit to avoid engine-queue bursts."""
        njt = 4 * g + 4
        qT = qT_g[g % 2]
        yT = yT_g[g % 2]

        def _finish_norm(p_, rcrows, is_f32):
            bdt = f32 if is_f32 else bf16
            bcb = bcp.tile([P, GN], bdt, tag="bcb" + ("f" if is_f32 else ""),
                           name=f"bcb{g}_{p_}")
            nc.gpsimd.partition_broadcast(bcb[0:64, :], rcrows[0][:])
            nc.vector.tensor_mul(yT[p_][0:64, :], yT[p_][0:64, :],
                                 bcb[0:64, :])
            bcb2 = bcp.tile([P, GN], bdt, tag="bcb" + ("f" if is_f32 else ""),
                            name=f"bcb2{g}_{p_}")
            nc.gpsimd.partition_broadcast(bcb2[0:64, :], rcrows[1][:])
            nc.sync.dma_start(bcb2[64:128, :], bcb2[0:64, :])
            nc.vector.tensor_mul(yT[p_][64:128, :],
                                 yT[p_][64:128, :], bcb2[64:128, :])

        pending_norm = None
        tfr = [0.0, 0.0]
        for p in range(4):
            yps = [psy.tile([65, GN], f32, tag="y", name=f"yps{g}_{p}_{s}")
                   for s in range(2)]
            # units: 4 diagonal tiles first (their mask latency hides
            # behind later work), then off-diagonal kv tile pairs with
            # fp8 DoubleRow PV.
            units = [("diag", j) for j in range(4 * g, njt)] + \
                    [("pair", m) for m in range(2 * g)]
            nu = len(units)
            prev = None

            def _pv(ui):
                kind, arg = units[ui]
                a_ = prev
                if kind == "diag":
                    for s in range(2):
                        nc.tensor.matmul(yps[s][:], vp4[:, arg, 2 * p + s, 0:65],
                                         a_[:, s * GN:(s + 1) * GN],
                                         start=(ui == 0), stop=(ui == nu - 1))
                else:
                    a8v = a_[:].rearrange("p (j x) -> p j x", j=2)
                    for s in range(2):
                        nc.tensor.matmul(
                            yps[s][:],
                            vp8[:, 2 * arg:2 * arg + 2, 2 * p + s, 0:65],
                            a8v[:, :, s * GN:(s + 1) * GN],
                            start=(ui == 0), stop=(ui == nu - 1),
                            perf_mode=DR, skip_group_check=True)

            for ui, (kind, arg) in enumerate(units):
                if kind == "diag":
                    j = arg
                    r = j - 4 * g
                    c0 = r * P
                    sc2 = pssc.tile([P, 2 * GN], f32, tag="sc",
                                    name=f"sc{g}_{p}_{j}")
                    for s in range(2):
                        hb = s * 64
                        nc.tensor.matmul(
                            sc2[:, s * GN + c0:(s + 1) * GN],
                            kT_t[p][hb:hb + 64, j * P:(j + 1) * P],
                            qT[p][hb:hb + 64, c0:GN],
                            start=True, stop=True)
                    a2 = attp.tile([P, 2 * GN], bf16, tag="att",
                                   name=f"att{g}_{p}_{j}")
                    sc2v = sc2[:].rearrange("p (s q) -> p s q", s=2)
                    a2v = a2[:].rearrange("p (s q) -> p s q", s=2)
                    if c0 > 0:
                        nc.gpsimd.memset(a2v[:, :, 0:c0], 0.0)
                    nc.scalar.activation(a2v[:, :, c0:GN], sc2v[:, :, c0:GN],
                                         AF.Exp, scale=1.0 / math.sqrt(D),
                                         bias=ebias_t[:, 0:1])
                    nc.vector.tensor_mul(a2[:, c0:c0 + P],
                                         a2[:, c0:c0 + P], dmask_t[:])
                    nc.vector.tensor_mul(a2[:, GN + c0:GN + c0 + P],
                                         a2[:, GN + c0:GN + c0 + P], dmask_t[:])
                    nxt = a2
                else:
                    m = arg
                    a8 = a8p.tile([P, 2 * 2 * GN], fp8, tag="a8",
                                  name=f"a8_{g}_{p}_{m}")
                    a8v = a8[:].rearrange("p (j x) -> p j x", j=2)
                    for jj in range(2):
                        j = 2 * m + jj
                        sc2 = pssc.tile([P, 2 * GN], f32, tag="sc",
                                        name=f"sc{g}_{p}_{j}")
                        for s in range(2):
                            hb = s * 64
                            nc.tensor.matmul(
                                sc2[:, s * GN:(s + 1) * GN],
                                kT_t[p][hb:hb + 64, j * P:(j + 1) * P],
                                qT[p][hb:hb + 64, :],
                                start=True, stop=True)
                        nc.scalar.activation(a8v[:, jj, :], sc2[:],
                                             AF.Exp, scale=1.0 / math.sqrt(D),
                                             bias=ebias_t[:, 0:1])
                    nxt = a8
                if ui > 0:
                    _pv(ui - 1)
                    prev = nxt
                else:
                    prev = nxt
                # static pacing model: emit interleaved work only while the
                # scalar (exp) frontier stays ahead of the PE frontier, so
                # filler never delays the exp pipeline's inputs
                if kind == "diag":
                    pe_u = 850
                    sc_u = (172 + 2 * (GN - (arg - 4 * g) * P)) / 1.2
                else:
                    pe_u, sc_u = 1180, 1994
                tfr[0] += pe_u
                tfr[1] = max(tfr[1], tfr[0]) + sc_u
                while work and tfr[1] - tfr[0] > work[0][0] + 300:
                    cost, fn = work.popleft()
                    fn()
                    tfr[0] += cost
            _pv(nu - 1)
            if pending_norm is not None:
                _finish_norm(*pending_norm)
            rcrows = []
            rs_p = rsp.tile([P, 8], f32, tag="rs", name=f"rs{g}_{p}")
            for s in range(2):
                nc.vector.tensor_copy(yT[p][s * 64:(s + 1) * 64, :],
                                      yps[s][0:64, :])
                rrow = rcp.tile([1, GN], f32, tag="rrow",
                                name=f"rrow{g}_{p}_{s}")
                nc.vector.tensor_copy(rrow[:], yps[s][64:65, :])
                nc.sync.dma_start(rs_p[:, s * 4:(s + 1) * 4], rrow[:])
            rc_p = rsp.tile([P, 8], f32, tag="rc", name=f"rcp{g}_{p}")
            nc.vector.reciprocal(rc_p[:], rs_p[:])
            rcb = rsp.tile([P, 8], bf16, tag="rcb", name=f"rcb{g}_{p}")
            nc.vector.tensor_copy(rcb[:], rc_p[:])
            for s in range(2):
                rcrow = rcp.tile([1, GN], bf16, tag="rcrow", bufs=2,
                                 name=f"rcrow{g}_{p}_{s}")
                nc.sync.dma_start(rcrow[:], rcb[:, s * 4:(s + 1) * 4])
                rcrows.append(rcrow)
            pending_norm = (p, rcrows, False)
        while work:
            work.popleft()[1]()
        _finish_norm(*pending_norm)

    # ================= main schedule =================
    from collections import deque

    ld = _loads(0)
    wv_loaded = [False]

    def _load_wvp():
        for k in range(KT):
            nc.sync.dma_start(wv_t[k][:], wvT[k * P:(k + 1) * P, :])
        for k in range(4):
            nc.sync.dma_start(wp_t[k][:], wpT[k * P:(k + 1) * P, :])

    _load_wvp()
    # g0 QKV emitted so attention(0) pair p's operands (f=p q-feats,
    # f=4+p k-feats) and V tiles complete early
    for p in range(4):
        _fstep(0, p, ld)
        _fstep(0, 4 + p, ld)
        _vstep(0, p, ld)

    next_ld = [None]

    def _qkv_items(g_, nl):
        items = []
        for p in range(4):
            items.append((1800, lambda p=p: _fstep(g_, p, nl)))
            items.append((1800, lambda p=p: _fstep(g_, 4 + p, nl)))
            items.append((2000, lambda p=p: _vstep(g_, p, nl)))
        return items

    for g in range(TG):
        work = deque()
        if g < TG - 1:
            next_ld[0] = _loads(g + 1)
            work.extend(_qkv_items(g + 1, next_ld[0]))
        if g >= 1:
            # previous group's projection, interleaved round-robin
            pitems = [(900, lambda tt=tt, n=n: _proj_chunk(g - 1, tt, n, psmm))
                      for tt in range(4) for n in range(2)]
            mixed = deque()
            wl = list(work)
            i = j = 0
            while i < len(wl) or j < len(pitems):
                if i < len(wl):
                    mixed.append(wl[i]); i += 1
                    if i % 3 == 0 and j < len(pitems):
                        mixed.append(pitems[j]); j += 1
                else:
                    mixed.append(pitems[j]); j += 1
            work = mixed
        _attention(g, work)

    # final projection (g3): 2-deep pipeline with the p=3 matmul last so
    # the first chunks' p0-p2 matmuls run under the last pair's norm chain
    g3 = TG - 1
    chunks = [(tt, n) for tt in range(4) for n in range(2)]
    o_tiles = {}
    for ci, (tt, n) in enumerate(chunks):
        o_ps = psy.tile([P, GN], f32, tag="y", name=f"opsF_{tt}_{n}")
        o_tiles[ci] = o_ps
        yTl = yT_g[g3 % 2]
        for i, p in enumerate((0, 1, 2)):
            nc.tensor.matmul(o_ps[:], yTl[p][:, tt * P:(tt + 1) * P],
                             wp_t[p][:, n * GN:(n + 1) * GN],
                             start=(i == 0), stop=False)
        if ci >= 1:
            _proj_fin(g3, chunks[ci - 1], o_tiles.pop(ci - 1))
    _proj_fin(g3, chunks[-1], o_tiles.pop(len(chunks) - 1))


def build_nc():
    nc = bacc.Bacc("TRN2", target_bir_lowering=False, debug=False,
                   num_devices=NCORES)
    xT = nc.dram_tensor("xT", [C, T], bf16, kind="ExternalInput").ap()
    wqkT = nc.dram_tensor("wqkT", [C, 1024], bf16, kind="ExternalInput").ap()
    wvT = nc.dram_tensor("wvT", [C, 512], bf16, kind="ExternalInput").ap()
    wpT = nc.dram_tensor("wpT", [512, 1024], bf16, kind="ExternalInput").ap()
    bqk = nc.dram_tensor("bqk", [P, 8], f32, kind="ExternalInput").ap()
    bv = nc.dram_tensor("bv", [1, 512], bf16, kind="ExternalInput").ap()
    cosT = nc.dram_tensor("cosT", [P, T], bf16, kind="ExternalInput").ap()
    sinT = nc.dram_tensor("sinT", [P, T], bf16, kind="ExternalInput").ap()
    dmask = nc.dram_tensor("dmask", [P, P], bf16, kind="ExternalInput").ap()
    onescol = nc.dram_tensor("onescol", [P, P], bf16, kind="ExternalInput").ap()
    outp = nc.dram_tensor("outp", [T, C], f16, kind="ExternalOutput").ap()
    with tile.TileContext(nc) as tc, \
            nc.allow_low_precision(reason="bf16/fp8 matmul operands"):
        with ExitStack() as ctx:
            _body(ctx, tc, xT, wqkT, wvT, wpT, bqk, bv, cosT, sinT,
                  dmask, onescol, outp)
    nc.compile()
    return nc


def _host_inputs(x, w_attn, b_attn, w_proj, cos, sin):
    """Build the 8 per-core input dicts."""
    dmask = np.triu(np.ones((P, P), np.float32))
    onescol = np.ones((P, P), np.float32)
    cosT2 = np.ascontiguousarray(
        np.concatenate([cos[0].T, cos[0].T], axis=0))      # [128, T]
    sinT2 = np.concatenate([sin[0].T, sin[0].T], axis=0)
    # rotate_half sign baked into sin: rows d<32 of each 64-row block get
    # the minus (tmp[0:32] = -u2*sin, tmp[32:64] = +u1*sin)
    sgn = np.where((np.arange(P) % 64) < 32, -1.0, 1.0).astype(np.float32)
    sinT2 = np.ascontiguousarray(sinT2 * sgn[:, None])
    dmask = dmask.astype(BF)
    onescol = onescol.astype(BF)
    cosT2 = cosT2.astype(BF)
    sinT2 = sinT2.astype(BF)

    in_maps = []
    for core in range(NCORES):
        b = core // 2
        hg = core % 2
        h0 = hg * HPC
        qrows = slice(h0 * D, (h0 + HPC) * D)              # 512 rows
        krows = slice(C + h0 * D, C + (h0 + HPC) * D)
        vrows = slice(2 * C + h0 * D, 2 * C + (h0 + HPC) * D)
        wqk = np.concatenate([w_attn[qrows], w_attn[krows]], axis=0)  # [1024, C]
        bqk_np = np.concatenate([b_attn[qrows], b_attn[krows]])       # [1024]
        in_maps.append({
            "xT": np.ascontiguousarray(x[b].T).astype(BF),             # [C, T]
            "wqkT": np.ascontiguousarray(wqk.T).astype(BF),            # [C, 1024]
            "wvT": np.ascontiguousarray(w_attn[vrows].T).astype(BF),   # [C, 512]
            "wpT": np.ascontiguousarray(
                w_proj[:, h0 * D:(h0 + HPC) * D].T).astype(BF),
            "bqk": np.ascontiguousarray(bqk_np.reshape(8, P).T),       # [128, 8]
            "bv": np.ascontiguousarray(
                b_attn[vrows].reshape(1, 512)).astype(BF),
            "cosT": cosT2,
            "sinT": sinT2,
            "dmask": dmask,
            "onescol": onescol,
        })
    return in_maps


def kernel(x, w_attn, b_attn, w_proj, b_proj, cos, sin):
    global _NC_CACHE
    x = np.asarray(x, np.float32)
    w_attn = np.asarray(w_attn, np.float32)
    b_attn = np.asarray(b_attn, np.float32)
    w_proj = np.asarray(w_proj, np.float32)
    b_proj = np.asarray(b_proj, np.float32)
    cos = np.asarray(cos, np.float32)
    sin = np.asarray(sin, np.float32)

    if _NC_CACHE is None:
        _NC_CACHE = build_nc()
    nc = _NC_CACHE
    in_maps = _host_inputs(x, w_attn, b_attn, w_proj, cos, sin)
    res = run_bass_kernel_spmd(nc, in_maps, core_ids=list(range(NCORES)))
    parts = [res.results[i]["outp"] for i in range(NCORES)]
    out = np.empty((B, T, C), np.float32)
    for b in range(B):
        out[b] = (parts[2 * b].astype(np.float32)
                  + parts[2 * b + 1].astype(np.float32) + b_proj)
    return out
